# revision 1
# baseline (speedup 1.0000x reference)
"""Cross-attention block kernel for Trainium2 (Bass/Tile), SPMD over 8 cores.

Sharding: data-parallel over batch B=8 -> one batch element per NeuronCore.
Per core:
  xn  = LayerNorm(xt) * w + b                      [4096, 128]
  cn  = LayerNorm(context) * cw + cb               [256, 768]
  q   = xn @ Wq                                    [4096, 512]  (8 heads x 64)
  k,v = cn @ Wkv (+ null kv row)                   [257, 512] each
  sim = q @ k^T / 8, softmax over keys (mask folded into v-side),
  out = attn @ v ; final = out @ Wout + bout + xn  [4096, 128]

Layout: keys on partitions for the softmax/AV stage (no transposes of the
attention matrix). simT = kT.T @ qT in PSUM, p = exp(simT/8) (no max
subtraction needed: |sim/8| <~ 2; masked keys contribute zero because the
v_aug rows are pre-multiplied by the mask). v_aug = [v*mask, mask] so one
AV matmul pair yields both the output rows and the softmax denominator;
the null key is folded in as 8 extra Wq columns (Wq_h @ k_null) plus one
K=1 AV matmul per head. The executor on this path has a large
per-instruction overhead, so ops are batched aggressively (whole-tensor
LayerNorm, 4-head division groups, packed PSUM->SBUF copies).
"""

import numpy as np

import concourse.bacc as bacc
import concourse.bass as bass
import concourse.mybir as mybir
import concourse.tile as tile
from concourse.bass_utils import run_bass_kernel_spmd
from concourse.masks import make_identity

B, XS, YS, C = 8, 64, 64, 128
CTX, N, H, D = 768, 256, 8, 64
HID = H * D          # 512
TOK = XS * YS        # 4096 tokens per batch element
TCH = 512            # tokens per chunk (PSUM bank free size in fp32)
NT = TOK // TCH      # 8 token chunks
NCORES = 8
F32 = mybir.dt.float32
F32R = mybir.dt.float32r
EPS = 1e-5
SCALE = D ** -0.5
Exp = mybir.ActivationFunctionType.Exp
Sqrt = mybir.ActivationFunctionType.Sqrt
Ident = mybir.ActivationFunctionType.Identity
Copy = mybir.ActivationFunctionType.Copy
SUB = mybir.AluOpType.subtract
MUL = mybir.AluOpType.mult
ADD = mybir.AluOpType.add


def build(n_iters: int = 1):
    nc = bacc.Bacc("TRN2", target_bir_lowering=False, debug=False,
                   num_devices=NCORES)

    xt_d = nc.dram_tensor("xt", [TOK, C], F32, kind="ExternalInput")
    ctx_d = nc.dram_tensor("context", [N, CTX], F32, kind="ExternalInput")
    mask_d = nc.dram_tensor("mask", [N], mybir.dt.uint8, kind="ExternalInput")
    nw_d = nc.dram_tensor("norm_w", [C], F32, kind="ExternalInput")
    nb_d = nc.dram_tensor("norm_b", [C], F32, kind="ExternalInput")
    cw_d = nc.dram_tensor("ctx_norm_w", [CTX], F32, kind="ExternalInput")
    cb_d = nc.dram_tensor("ctx_norm_b", [CTX], F32, kind="ExternalInput")
    wq_d = nc.dram_tensor("Wq", [C, HID], F32, kind="ExternalInput")
    wkv_d = nc.dram_tensor("Wkv", [CTX, 2 * HID], F32, kind="ExternalInput")
    nkv_d = nc.dram_tensor("null_kv", [2, D], F32, kind="ExternalInput")
    wout_d = nc.dram_tensor("Wout", [HID, C], F32, kind="ExternalInput")
    bout_d = nc.dram_tensor("bout", [C], F32, kind="ExternalInput")
    out_d = nc.dram_tensor("out", [TOK, C], F32, kind="ExternalOutput")

    def bc_ap(handle, n_part, n_free):
        # broadcast a [n_free] DRAM vector across n_part partitions
        return bass.AP(handle, 0, [[0, n_part], [1, n_free]])

    def col_ap(handle, n_part, row=0):
        # load a [n_free] DRAM vector into n_part partitions x 1
        return bass.AP(handle, row * n_part, [[1, n_part], [1, 1]])

    with tile.TileContext(nc) as tc:
        with (
            tc.tile_pool(name="const", bufs=1) as const,
            tc.tile_pool(name="wides", bufs=1) as wides,
            tc.tile_pool(name="persist", bufs=1) as persist,
            tc.tile_pool(name="work", bufs=2) as work,
            tc.tile_pool(name="work2", bufs=2) as work2,
            tc.tile_pool(name="small", bufs=2) as small,
            # PSUM budget (8 banks): ps 2 + po4 4 + aux 2
            tc.tile_pool(name="pbig", bufs=1, space=bass.MemorySpace.PSUM) as pbig,
            tc.tile_pool(name="pout", bufs=1, space=bass.MemorySpace.PSUM) as pout,
            tc.tile_pool(name="paux", bufs=1, space=bass.MemorySpace.PSUM) as paux,
        ):
            ident = const.tile([128, 128], F32)
            make_identity(nc, ident)
            identr = const.tile([128, 128], F32R)
            nc.vector.tensor_copy(out=identr, in_=ident)
            eps_t = const.tile([128, 1], F32)
            nc.vector.memset(eps_t, EPS)

            for _ in range(n_iters):
                # ---- weights (gpsimd casting DMAs round fp32 -> f32r) ----
                wq_sb = wides.tile([C, HID], F32R, tag="wq")
                nc.gpsimd.dma_start(out=wq_sb, in_=wq_d.ap())
                wkv_sb = wides.tile([128, 6, 2 * HID], F32R, tag="wkv")
                nc.gpsimd.dma_start(
                    out=wkv_sb,
                    in_=bass.AP(wkv_d, 0, [[2 * HID, 128], [128 * 2 * HID, 6],
                                           [1, 2 * HID]]))
                # Wout as [d=64, head, C] so K=64 matmuls start at partition 0
                wout_sb = wides.tile([D, H, C], F32R, tag="wout")
                nc.gpsimd.dma_start(
                    out=wout_sb,
                    in_=bass.AP(wout_d, 0, [[C, D], [D * C, H], [1, C]]))
                nw_bc = wides.tile([128, C], F32, tag="nw")
                nc.sync.dma_start(out=nw_bc, in_=bc_ap(nw_d, 128, C))
                nb_bc = wides.tile([128, C], F32, tag="nb")
                nc.sync.dma_start(out=nb_bc, in_=bc_ap(nb_d, 128, C))
                cw_bc = wides.tile([128, CTX], F32, tag="cw")
                nc.sync.dma_start(out=cw_bc, in_=bc_ap(cw_d, 128, CTX))
                cb_bc = wides.tile([128, CTX], F32, tag="cb")
                nc.sync.dma_start(out=cb_bc, in_=bc_ap(cb_d, 128, CTX))
                bout_sb = wides.tile([C, 1], F32, tag="bout")
                nc.sync.dma_start(out=bout_sb, in_=col_ap(bout_d, C))

                mask8 = small.tile([128, 2], mybir.dt.uint8, tag="m8")
                nc.sync.dma_start(out=mask8,
                                  in_=bass.AP(mask_d, 0, [[1, 128], [128, 2]]))
                maskf = wides.tile([128, 2], F32, tag="mf")
                nc.vector.tensor_copy(out=maskf, in_=mask8)

                # ---- Wq^T, then the 8 null-q columns wqnull = Wq_h @ k_null
                wqT = wides.tile([128, 4, 128], F32R, tag="wqT")
                for m in range(2):
                    ptw = paux.tile([128, 2, 128], F32R, tag="aux")
                    nc.tensor.transpose(ptw[:, 0, :],
                                        wq_sb[:, (2 * m) * 128:(2 * m + 1) * 128],
                                        identr)
                    nc.tensor.transpose(ptw[:, 1, :],
                                        wq_sb[:, (2 * m + 1) * 128:(2 * m + 2) * 128],
                                        identr)
                    nc.scalar.copy(out=wqT[:, 2 * m:2 * m + 2, :], in_=ptw)
                kblk = wides.tile([128, 4, H], F32, tag="kblk")
                nc.vector.memset(kblk, 0.0)
                for h in range(H):
                    hb = (h % 2) * 64
                    nc.sync.dma_start(
                        out=kblk[hb:hb + 64, h // 2, h:h + 1],
                        in_=col_ap(nkv_d, D, 0))
                kblk_r = wides.tile([128, 4, H], F32R, tag="kblk_r")
                nc.vector.tensor_copy(out=kblk_r, in_=kblk)
                pwn = paux.tile([H, 128], F32, tag="aux")
                for m in range(4):
                    nc.tensor.matmul(pwn, kblk_r[:, m, :], wqT[:, m, :],
                                     start=(m == 0), stop=(m == 3))
                wqnT = small.tile([H, C], F32, tag="wqnT")
                nc.scalar.copy(out=wqnT, in_=pwn)
                pwt = paux.tile([128, 128], F32, tag="aux")
                nc.tensor.matmul(pwt, wqnT, ident[0:H, :],
                                 is_transpose=True, start=True, stop=True)
                wqnull = wides.tile([C, H], F32R, tag="wqnull")
                nc.scalar.copy(out=wqnull, in_=pwt[:, 0:H])

                # null-value row for the AV matmul: [1, 65] = [v_null, 1]
                vnull = small.tile([1, D + 1], F32, tag="vnull")
                nc.sync.dma_start(out=vnull[:, 0:D],
                                  in_=bass.AP(nkv_d, D, [[1, 1], [1, D]]))
                nc.vector.memset(vnull[:, D:D + 1], 1.0)
                vnull_r = small.tile([1, D + 1], F32R, tag="vnull_r")
                nc.vector.tensor_copy(out=vnull_r, in_=vnull)

                # ---- context LN (batched over both 128-token groups) ----
                cn2 = persist.tile([128, 2, CTX], F32, tag="cn2")
                nc.sync.dma_start(
                    out=cn2, in_=ctx_d.ap().rearrange("(g p) c -> p g c", p=128))
                sqc = persist.tile([128, 2, CTX], F32, tag="sqc")
                nc.vector.tensor_mul(out=sqc, in0=cn2, in1=cn2)
                cst = small.tile([128, 2, 2], F32, tag="cst")
                nc.vector.reduce_sum(out=cst[:, :, 0], in_=cn2,
                                     axis=mybir.AxisListType.X)
                nc.vector.reduce_sum(out=cst[:, :, 1], in_=sqc,
                                     axis=mybir.AxisListType.X)
                cmu = small.tile([128, 2], F32, tag="cmu")
                nc.scalar.activation(out=cmu, in_=cst[:, :, 0], func=Copy,
                                     scale=1.0 / CTX)
                cvar = small.tile([128, 2], F32, tag="cvar")
                nc.vector.tensor_mul(out=cvar, in0=cmu, in1=cmu)
                nc.vector.scalar_tensor_tensor(out=cvar, in0=cst[:, :, 1],
                                               scalar=1.0 / CTX, in1=cvar,
                                               op0=MUL, op1=SUB)
                csd = small.tile([128, 2], F32, tag="csd")
                nc.scalar.activation(out=csd, in_=cvar, func=Sqrt, bias=eps_t)
                nc.vector.reciprocal(out=csd, in_=csd)
                nc.vector.tensor_tensor(
                    out=cn2, in0=cn2,
                    in1=cmu.unsqueeze(2).broadcast_to((128, 2, CTX)), op=SUB)
                nc.vector.tensor_tensor(
                    out=cn2, in0=cn2,
                    in1=csd.unsqueeze(2).broadcast_to((128, 2, CTX)), op=MUL)
                nc.gpsimd.tensor_tensor(
                    out=cn2, in0=cn2,
                    in1=cw_bc.unsqueeze(1).broadcast_to((128, 2, CTX)), op=MUL)
                nc.gpsimd.tensor_tensor(
                    out=cn2, in0=cn2,
                    in1=cb_bc.unsqueeze(1).broadcast_to((128, 2, CTX)), op=ADD)
                cnT = wides.tile([128, 6, N], F32R, tag="cnT")
                for rr0 in range(0, 6, 2):
                    pt4 = paux.tile([128, 2, 2, 128], F32, tag="aux")
                    for j in range(2):
                        for t in range(2):
                            nc.tensor.transpose(
                                pt4[:, j, t, :],
                                cn2[:, t, (rr0 + j) * 128:(rr0 + j + 1) * 128],
                                ident)
                    nc.scalar.copy(out=cnT[:, rr0:rr0 + 2, :], in_=pt4)

                # ---- k,v = cn @ Wkv in [keys, HID] layout (aligned lhsT),
                # then kT per head via PE transposes; v_aug padded to stride 128
                kt_all = wides.tile([128, 4, N], F32R, tag="kt_all")
                va = wides.tile([128, 2, H, D + 1], F32, tag="va")
                for kc in range(2):
                    pkv = paux.tile([128, HID], F32, tag="aux")
                    for rr in range(6):
                        nc.tensor.matmul(
                            pkv, cnT[:, rr, kc * 128:(kc + 1) * 128],
                            wkv_sb[:, rr, 0:HID],
                            start=(rr == 0), stop=(rr == 5))
                    ksb = work.tile([128, HID], F32, tag="ksb")
                    nc.scalar.copy(out=ksb, in_=pkv)
                    ptk4 = paux.tile([128, 4, 128], F32, tag="aux")
                    for c in range(4):
                        nc.tensor.transpose(ptk4[:, c, :],
                                            ksb[:, c * 128:(c + 1) * 128], ident)
                    nc.scalar.copy(out=kt_all[:, :, kc * 128:(kc + 1) * 128],
                                   in_=ptk4)
                    pv = paux.tile([128, HID], F32, tag="aux")
                    for rr in range(6):
                        nc.tensor.matmul(
                            pv, cnT[:, rr, kc * 128:(kc + 1) * 128],
                            wkv_sb[:, rr, HID:2 * HID],
                            start=(rr == 0), stop=(rr == 5))
                    nc.vector.tensor_copy(
                        out=va[:, kc, :, 0:D],
                        in_=pv.rearrange("p (h d) -> p h d", h=H))
                nc.vector.memset(va[:, :, :, D:D + 1], 1.0)
                for kc in range(2):
                    nc.gpsimd.tensor_scalar_mul(va[:, kc], va[:, kc],
                                                maskf[:, kc:kc + 1])
                # rounded copy, padded so each head's lhsT slice is 128-aligned
                va_r = wides.tile([128, 2, H, 128], F32R, tag="va_r")
                nc.vector.tensor_copy(out=va_r[:, :, :, 0:D + 1], in_=va)

                # ---- xt LN (batched over all 32 groups) -> xnT [C, 4096] ----
                x_all = persist.tile([128, 32, C], F32, tag="p16b")
                nc.sync.dma_start(
                    out=x_all, in_=xt_d.ap().rearrange("(g p) c -> p g c", p=128))
                sq = persist.tile([128, 32, C], F32, tag="s16")
                nc.vector.tensor_mul(out=sq, in0=x_all, in1=x_all)
                xst = small.tile([128, 32, 2], F32, tag="xst")
                nc.vector.reduce_sum(out=xst[:, :, 0], in_=x_all,
                                     axis=mybir.AxisListType.X)
                nc.vector.reduce_sum(out=xst[:, :, 1], in_=sq,
                                     axis=mybir.AxisListType.X)
                xmu = small.tile([128, 32], F32, tag="xmu")
                nc.scalar.activation(out=xmu, in_=xst[:, :, 0], func=Copy,
                                     scale=1.0 / C)
                xvar = small.tile([128, 32], F32, tag="xvar")
                nc.vector.tensor_mul(out=xvar, in0=xmu, in1=xmu)
                nc.vector.scalar_tensor_tensor(out=xvar, in0=xst[:, :, 1],
                                               scalar=1.0 / C, in1=xvar,
                                               op0=MUL, op1=SUB)
                xsd = small.tile([128, 32], F32, tag="xsd")
                nc.scalar.activation(out=xsd, in_=xvar, func=Sqrt, bias=eps_t)
                nc.vector.reciprocal(out=xsd, in_=xsd)
                nc.vector.tensor_tensor(
                    out=x_all, in0=x_all,
                    in1=xmu.unsqueeze(2).broadcast_to((128, 32, C)), op=SUB)
                nc.vector.tensor_tensor(
                    out=x_all, in0=x_all,
                    in1=xsd.unsqueeze(2).broadcast_to((128, 32, C)), op=MUL)
                nc.gpsimd.tensor_tensor(
                    out=x_all, in0=x_all,
                    in1=nw_bc.unsqueeze(1).broadcast_to((128, 32, C)), op=MUL)
                nc.gpsimd.tensor_tensor(
                    out=x_all, in0=x_all,
                    in1=nb_bc.unsqueeze(1).broadcast_to((128, 32, C)), op=ADD)
                xnT = wides.tile([C, TOK], F32R, tag="xnT")
                for t4 in range(8):
                    pt4 = paux.tile([128, 4, 128], F32, tag="aux")
                    for j in range(4):
                        nc.tensor.transpose(pt4[:, j, :], x_all[:, 4 * t4 + j, :],
                                            ident)
                    nc.scalar.copy(out=xnT[:, t4 * 512:(t4 + 1) * 512], in_=pt4)

                # ---- attention per 512-token chunk ----
                for t in range(NT):
                    tsl = slice(t * TCH, (t + 1) * TCH)
                    qT = work2.tile([128, 4, TCH], F32R, tag="qT")
                    for m in range(2):
                        pq = pbig.tile([128, 2, TCH], F32, tag="ps")
                        nc.tensor.matmul(pq[:, 0, :],
                                         wq_sb[:, (2 * m) * 128:(2 * m + 1) * 128],
                                         xnT[:, tsl], start=True, stop=True)
                        nc.tensor.matmul(pq[:, 1, :],
                                         wq_sb[:, (2 * m + 1) * 128:(2 * m + 2) * 128],
                                         xnT[:, tsl], start=True, stop=True)
                        nc.scalar.copy(out=qT[:, 2 * m:2 * m + 2, :], in_=pq)
                    # null-key sims for all 8 heads at once
                    pq5 = paux.tile([128, TCH], F32, tag="aux")
                    nc.tensor.matmul(pq5[0:H, :], wqnull, xnT[:, tsl],
                                     start=True, stop=True)
                    pn_exp = small.tile([H, TCH], F32R, tag="pn_exp")
                    nc.scalar.activation(out=pn_exp, in_=pq5[0:H, :], func=Exp,
                                         scale=SCALE)
                    # rearrange to one partition so each head's row is a
                    # base-0 matmul rhs
                    pn1 = persist.tile([1, H, TCH], F32R, tag="s16")
                    nc.sync.dma_start(out=pn1, in_=pn_exp)
                    outT = persist.tile([D, H, TCH], F32R, tag="p16b")
                    for qd in range(2):
                        po4 = pout.tile([D + 1, 4, TCH], F32, tag="po4")
                        for hh in range(4):
                            h = qd * 4 + hh
                            hb = (h % 2) * 64
                            qh = qT[hb:hb + 64, h // 2, :]
                            ps = pbig.tile([128, 2, TCH], F32, tag="ps")
                            nc.tensor.matmul(ps[:, 0, :],
                                             kt_all[hb:hb + 64, h // 2, 0:128],
                                             qh, start=True, stop=True)
                            nc.tensor.matmul(ps[:, 1, :],
                                             kt_all[hb:hb + 64, h // 2, 128:256],
                                             qh, start=True, stop=True)
                            pe = work.tile([128, 2, TCH], F32R, tag="pexp")
                            nc.scalar.activation(out=pe, in_=ps, func=Exp,
                                                 scale=SCALE)
                            nc.tensor.matmul(po4[:, hh, :], va_r[:, 0, h, 0:D + 1],
                                             pe[:, 0, :], start=True, stop=False)
                            nc.tensor.matmul(po4[:, hh, :], va_r[:, 1, h, 0:D + 1],
                                             pe[:, 1, :], start=False, stop=False)
                            nc.tensor.matmul(po4[:, hh, :], vnull_r, pn1[:, h, :],
                                             start=False, stop=True)
                        # one reciprocal / broadcast / divide for the 4 heads
                        rc4 = small.tile([1, 4, TCH], F32, tag="rc4")
                        nc.vector.reciprocal(out=rc4, in_=po4[D:D + 1, :, :])
                        rb4 = persist.tile([D, 4, TCH], F32, tag="rb4")
                        nc.gpsimd.partition_broadcast(rb4, rc4)
                        nc.vector.tensor_mul(out=outT[:, 4 * qd:4 * qd + 4, :],
                                             in0=po4[0:D, :, :], in1=rb4)

                    # ---- final projection + bias + residual ----
                    pf = paux.tile([C, TCH], F32, tag="aux")
                    for h in range(H):
                        nc.tensor.matmul(pf, wout_sb[:, h, :], outT[:, h, :],
                                         start=(h == 0), stop=(h == H - 1))
                    fT = work.tile([C, TCH], F32, tag="fT")
                    nc.scalar.activation(out=fT, in_=pf, func=Ident,
                                         bias=bout_sb)
                    nc.gpsimd.tensor_add(out=fT, in0=fT,
                                         in1=xnT[:, tsl].bitcast(F32))
                    pt4 = paux.tile([128, 4, 128], F32, tag="aux")
                    for sblk in range(4):
                        nc.tensor.transpose(pt4[:, sblk, :],
                                            fT[:, sblk * 128:(sblk + 1) * 128],
                                            ident)
                    fo = work.tile([128, 4, C], F32, tag="fo")
                    nc.scalar.copy(out=fo, in_=pt4)
                    orows = out_d.ap()[t * TCH:(t + 1) * TCH, :]
                    nc.sync.dma_start(
                        out=orows.rearrange("(s p) c -> p s c", p=128), in_=fo)

    nc.compile()
    return nc


_CACHE = {}


def get_nc(n_iters: int = 1):
    if n_iters not in _CACHE:
        _CACHE[n_iters] = build(n_iters)
    return _CACHE[n_iters]


def make_in_maps(xt, context, mask, norm_w, norm_b, ctx_norm_w, ctx_norm_b,
                 Wq, Wkv, null_kv, Wout, bout):
    xt = np.asarray(xt, dtype=np.float32).reshape(B, TOK, C)
    context = np.asarray(context, dtype=np.float32)
    mask8 = np.asarray(mask).astype(np.uint8)
    shared = {
        "norm_w": np.asarray(norm_w, np.float32),
        "norm_b": np.asarray(norm_b, np.float32),
        "ctx_norm_w": np.asarray(ctx_norm_w, np.float32),
        "ctx_norm_b": np.asarray(ctx_norm_b, np.float32),
        "Wq": np.asarray(Wq, np.float32),
        "Wkv": np.asarray(Wkv, np.float32),
        "null_kv": np.asarray(null_kv, np.float32),
        "Wout": np.asarray(Wout, np.float32),
        "bout": np.asarray(bout, np.float32),
    }
    return [
        {"xt": xt[b], "context": context[b], "mask": mask8[b], **shared}
        for b in range(B)
    ]


def kernel(xt, context, mask, norm_w, norm_b, ctx_norm_w, ctx_norm_b,
           Wq, Wkv, null_kv, Wout, bout):
    nc = get_nc(1)
    in_maps = make_in_maps(xt, context, mask, norm_w, norm_b, ctx_norm_w,
                           ctx_norm_b, Wq, Wkv, null_kv, Wout, bout)
    res = run_bass_kernel_spmd(nc, in_maps, core_ids=list(range(NCORES)))
    out = np.stack([res.results[b]["out"] for b in range(B)], axis=0)
    return out.reshape(B, XS, YS, C).astype(np.float32)



# revision 2
# speedup vs baseline: 44.2908x; 44.2908x over previous
"""Cross-attention block kernel v2 for Trainium2 (Bass/Tile), SPMD 8 cores.

Data-parallel over batch B=8 -> one batch element per core.

v2 is organized around DMA descriptor cost (measured ~1ms per 128x4B or
512x512B descriptor pattern on this path): every large DRAM<->SBUF transfer
is contiguous per partition, small vectors are loaded as single-descriptor
rows and moved into column/partition form with one batched PE transpose,
weights are bit-cast to f32r instead of gpsimd casting DMAs, and the output
is accumulated in SBUF and stored with one DMA.

Layouts (per core):
  x_all[p, g, c]   = xt[p*32+g, c]          (tokens p-major, 128x16KB DMA)
  out_sb[p, g, c]  = out[p*32+g, c]         (one 128x16KB store)
  cn2[p, g, c]     = context[g*128+p, c]    (keys block-major, 256x3KB)
  wkv_sb[p, r, :]  = Wkv[6p+r, :]           (contiguous, 128x24KB)
  wraw[p, r, c]    = Wout[4p+r, c]          (contiguous, 128x2KB)
  cnT[j, r, g, k]  = cn_norm[key g*128+k, ctx 6j+r]  (stride-6 PE transposes)
  wout_hd[d, h, c] = Wout[h*64+d, c]        (PE double transpose from wraw)

LayerNorm gamma/beta for xt are applied after the transpose as per-partition
activation scale/bias fused into the PSUM eviction. Softmax needs no max
subtraction (|sim|/8 <= ~1.2); the key mask is folded into the v side
(va rows pre-multiplied by mask, ones column accumulates the denominator).
The null key enters via 8 extra Wq columns (wqnull = Wq_h @ k_null) and one
K=1 AV matmul per head.
"""

import numpy as np

import concourse.bacc as bacc
import concourse.bass as bass
import concourse.mybir as mybir
import concourse.tile as tile
from concourse.masks import make_identity

B, XS, YS, C = 8, 64, 64, 128
CTX, N, H, D = 768, 256, 8, 64
HID = H * D          # 512
TOK = XS * YS        # 4096
TCH = 512            # tokens per attention chunk
NT = TOK // TCH      # 8 chunks
GP = TOK // 128      # 32 token groups per partition
NCORES = 8
F32 = mybir.dt.float32
F32R = mybir.dt.float32r
EPS = 1e-5
SCALE = D ** -0.5
Exp = mybir.ActivationFunctionType.Exp
Sqrt = mybir.ActivationFunctionType.Sqrt
Ident = mybir.ActivationFunctionType.Identity
Copy = mybir.ActivationFunctionType.Copy
SUB = mybir.AluOpType.subtract
MUL = mybir.AluOpType.mult
ADD = mybir.AluOpType.add


def build(n_iters: int = 1):
    nc = bacc.Bacc("TRN2", target_bir_lowering=False, debug=False,
                   num_devices=NCORES)

    xt_d = nc.dram_tensor("xt", [TOK, C], F32, kind="ExternalInput")
    ctx_d = nc.dram_tensor("context", [N, CTX], F32, kind="ExternalInput")
    mask_d = nc.dram_tensor("mask", [N], mybir.dt.uint8, kind="ExternalInput")
    nw_d = nc.dram_tensor("norm_w", [C], F32, kind="ExternalInput")
    nb_d = nc.dram_tensor("norm_b", [C], F32, kind="ExternalInput")
    cw_d = nc.dram_tensor("ctx_norm_w", [CTX], F32, kind="ExternalInput")
    cb_d = nc.dram_tensor("ctx_norm_b", [CTX], F32, kind="ExternalInput")
    wq_d = nc.dram_tensor("Wq", [C, HID], F32, kind="ExternalInput")
    wkv_d = nc.dram_tensor("Wkv", [CTX, 2 * HID], F32, kind="ExternalInput")
    nkv_d = nc.dram_tensor("null_kv", [2, D], F32, kind="ExternalInput")
    wout_d = nc.dram_tensor("Wout", [HID, C], F32, kind="ExternalInput")
    bout_d = nc.dram_tensor("bout", [C], F32, kind="ExternalInput")
    out_d = nc.dram_tensor("out", [TOK, C], F32, kind="ExternalOutput")

    def row_ap(handle, n):
        return bass.AP(handle, 0, [[0, 1], [1, n]])

    with tile.TileContext(nc) as tc:
        with (
            tc.tile_pool(name="const", bufs=1) as const,
            tc.tile_pool(name="wides", bufs=1) as wides,
            tc.tile_pool(name="persist", bufs=1) as persist,
            tc.tile_pool(name="work", bufs=2) as work,
            tc.tile_pool(name="work1", bufs=1) as work1,
            tc.tile_pool(name="small", bufs=2) as small,
            # PSUM: psim 2 slots x 2 banks + pacc 1 slot x 4 banks = 8 banks
            tc.tile_pool(name="psim", bufs=2, space=bass.MemorySpace.PSUM) as psim,
            tc.tile_pool(name="pacc", bufs=1, space=bass.MemorySpace.PSUM) as pacc,
        ):
            ident = const.tile([128, 128], F32)
            make_identity(nc, ident)
            identr = const.tile([128, 128], F32R)
            nc.vector.tensor_copy(out=identr, in_=ident)
            eps_t = const.tile([128, 1], F32)
            nc.vector.memset(eps_t, EPS)
            ones_f = const.tile([1, 128], F32)
            nc.vector.memset(ones_f, 1.0)
            onesr = const.tile([1, 128], F32R)
            nc.vector.tensor_copy(out=onesr, in_=ones_f)

            for _ in range(n_iters):
                # ================= DMAs =====================================
                x_all = persist.tile([128, GP, C], F32, tag="x_all")
                nc.sync.dma_start(
                    out=x_all, in_=xt_d.ap().rearrange("(p g) c -> p g c", p=128))
                wq_sb = wides.tile([C, HID], F32R, tag="wq")
                nc.gpsimd.dma_start(out=wq_sb, in_=wq_d.ap())
                wkv_sb = wides.tile([128, 6, 2 * HID], F32R, tag="wkv")
                nc.gpsimd.dma_start(
                    out=wkv_sb,
                    in_=wkv_d.ap().rearrange("(p r) c -> p r c", p=128))
                wraw = wides.tile([128, 4, C], F32, tag="wraw")
                nc.sync.dma_start(
                    out=wraw, in_=wout_d.ap().rearrange("(p r) c -> p r c", p=128))
                cn2 = persist.tile([128, 2, CTX], F32, tag="cn2")
                nc.sync.dma_start(
                    out=cn2, in_=ctx_d.ap().rearrange("(g p) c -> p g c", p=128))
                mrow8 = wides.tile([1, N], mybir.dt.uint8, tag="mrow8")
                nc.sync.dma_start(out=mrow8, in_=row_ap(mask_d, N))
                nkrow = wides.tile([1, 2 * D], F32, tag="nkrow")
                nc.sync.dma_start(out=nkrow, in_=row_ap(nkv_d, 2 * D))
                cwrow_f = wides.tile([1, 2, 384], F32, tag="cwrow_f")
                nc.sync.dma_start(out=cwrow_f, in_=row_ap(cw_d, CTX))
                cwrow = wides.tile([1, 2, 384], F32R, tag="cwrow")
                nc.vector.tensor_copy(out=cwrow, in_=cwrow_f)
                cbrow_f = wides.tile([1, 2, 384], F32, tag="cbrow_f")
                nc.sync.dma_start(out=cbrow_f, in_=row_ap(cb_d, CTX))
                cbrow = wides.tile([1, 2, 384], F32R, tag="cbrow")
                nc.vector.tensor_copy(out=cbrow, in_=cbrow_f)

                # ====== small-vector plumbing: one batched PE transpose =====
                # rows7: 0=norm_w 1=norm_b 2=bout 3=k_null(lo) 4=k_null(hi)
                #        5=mask[0:128] 6=mask[128:256]
                rows7 = wides.tile([7, 128], F32, tag="rows7")
                nc.sync.dma_start(out=rows7[0:1, :], in_=row_ap(nw_d, C))
                nc.sync.dma_start(out=rows7[1:2, :], in_=row_ap(nb_d, C))
                nc.sync.dma_start(out=rows7[2:3, :], in_=row_ap(bout_d, C))
                b2 = wides.tile([1, 2, 128], F32, tag="b2")
                nc.vector.memset(b2, 0.0)
                nc.vector.tensor_copy(out=b2[:, 0, 0:D], in_=nkrow[:, 0:D])
                nc.vector.tensor_copy(out=b2[:, 1, D:2 * D], in_=nkrow[:, 0:D])
                nc.sync.dma_start(out=rows7[3:5, :], in_=b2)
                mrowf = wides.tile([1, 2, 128], F32, tag="mrowf")
                nc.vector.tensor_copy(
                    out=mrowf, in_=mrow8.rearrange("p (g c) -> p g c", g=2))
                nc.sync.dma_start(out=rows7[5:7, :], in_=mrowf)
                pcols = psim.tile([128, 128], F32, tag="ps")
                nc.tensor.matmul(pcols, rows7, ident[0:7, :],
                                 is_transpose=True, start=True, stop=True)
                cols = wides.tile([128, 7], F32, tag="cols")
                nc.scalar.copy(out=cols, in_=pcols[:, 0:7])
                nw_col, nb_col, bout_col = cols[:, 0:1], cols[:, 1:2], cols[:, 2:3]
                knull2 = cols[:, 3:5]
                maskf = cols[:, 5:7]

                # cw/cb broadcast tiles via PE ones-outer-product
                pcw = pacc.tile([128, 2, 512], F32, tag="acc")
                for half in range(2):
                    nc.tensor.matmul(pcw[:, half, 0:384], onesr,
                                     cwrow[:, half, :], start=True, stop=True)
                cw_bc = wides.tile([128, 2, 384], F32, tag="cw_bc")
                nc.scalar.copy(out=cw_bc, in_=pcw[:, :, 0:384])
                pcb = pacc.tile([128, 2, 512], F32, tag="acc")
                for half in range(2):
                    nc.tensor.matmul(pcb[:, half, 0:384], onesr,
                                     cbrow[:, half, :], start=True, stop=True)
                cb_bc = wides.tile([128, 2, 384], F32, tag="cb_bc")
                nc.scalar.copy(out=cb_bc, in_=pcb[:, :, 0:384])

                # null-k block-diagonal [128, 4, H] (f32 bits into f32r tile)
                kblk_f = wides.tile([128, 4, H], F32, tag="kblk_f")
                nc.vector.memset(kblk_f, 0.0)
                for m in range(4):
                    nc.vector.tensor_copy(
                        out=kblk_f[:, m, 2 * m:2 * m + 2], in_=knull2)
                kblk = wides.tile([128, 4, H], F32R, tag="kblk")
                nc.vector.tensor_copy(out=kblk, in_=kblk_f)
                vrow = wides.tile([1, D + 1], F32, tag="vrow")
                nc.vector.tensor_copy(out=vrow[:, 0:D], in_=nkrow[:, D:2 * D])
                nc.vector.memset(vrow[:, D:D + 1], 1.0)
                # vnull8[:, h, :] = e_h (x) [v_null, 1] for K=8 null AV matmuls
                vnull8_f = wides.tile([H, H, D + 1], F32, tag="vnull8_f")
                nc.vector.memset(vnull8_f, 0.0)
                for h in range(H):
                    nc.sync.dma_start(out=vnull8_f[h:h + 1, h, :], in_=vrow)
                vnull8 = wides.tile([H, H, D + 1], F32R, tag="vnull8")
                nc.vector.tensor_copy(out=vnull8, in_=vnull8_f)

                # wqT; wqnull[C, H] = Wq @ blockdiag(k_null)
                pwq = pacc.tile([128, 4, 128], F32R, tag="acc")
                for m in range(4):
                    nc.tensor.transpose(pwq[:, m, :],
                                        wq_sb[:, m * 128:(m + 1) * 128], identr)
                wqT = wides.tile([128, 4, 128], F32R, tag="wqT")
                nc.scalar.copy(out=wqT, in_=pwq)
                pwn = psim.tile([H, 128], F32, tag="ps")
                for m in range(4):
                    nc.tensor.matmul(pwn, kblk[:, m, :], wqT[:, m, :],
                                     start=(m == 0), stop=(m == 3))
                wqnT = wides.tile([H, C], F32, tag="wqnT")
                nc.scalar.copy(out=wqnT, in_=pwn)
                pwt = psim.tile([128, 128], F32, tag="ps")
                nc.tensor.matmul(pwt, wqnT, ident[0:H, :],
                                 is_transpose=True, start=True, stop=True)
                wqnull = wides.tile([C, H], F32R, tag="wqnull")
                nc.scalar.copy(out=wqnull, in_=pwt[:, 0:H])

                # Wout p-major -> block-major [d, h, c] via double transpose
                pw1 = pacc.tile([128, 4, C], F32, tag="acc")
                for r in range(4):
                    nc.tensor.transpose(pw1[:, r, :], wraw[:, r, :], ident)
                # wT[c, hid] = Wout[hid, c]: interleave (r, p) -> 4p+r
                wT = wides.tile([128, HID], F32, tag="wT")
                for r in range(4):
                    nc.scalar.copy(out=wT[:, r::4], in_=pw1[:, r, :])
                pw2 = pacc.tile([D, H, C], F32, tag="acc")
                for h in range(H):
                    nc.tensor.transpose(pw2[:, h, :],
                                        wT[:, h * D:(h + 1) * D], ident)
                wout_hd = wides.tile([D, H, C], F32R, tag="wout_hd")
                nc.scalar.copy(out=wout_hd, in_=pw2)

                # ================= context LayerNorm + cnT + k/v ============
                sqc = persist.tile([128, 2, CTX], F32, tag="rb4")
                nc.vector.tensor_mul(out=sqc, in0=cn2, in1=cn2)
                cst = small.tile([128, 2, 2], F32, tag="cst")
                nc.vector.reduce_sum(out=cst[:, :, 0], in_=cn2,
                                     axis=mybir.AxisListType.X)
                nc.vector.reduce_sum(out=cst[:, :, 1], in_=sqc,
                                     axis=mybir.AxisListType.X)
                cmu = small.tile([128, 2], F32, tag="cmu")
                nc.scalar.activation(out=cmu, in_=cst[:, :, 0], func=Copy,
                                     scale=1.0 / CTX)
                cvar = small.tile([128, 2], F32, tag="cvar")
                nc.vector.tensor_mul(out=cvar, in0=cmu, in1=cmu)
                nc.vector.scalar_tensor_tensor(out=cvar, in0=cst[:, :, 1],
                                               scalar=1.0 / CTX, in1=cvar,
                                               op0=MUL, op1=SUB)
                csd = small.tile([128, 2], F32, tag="csd")
                nc.scalar.activation(out=csd, in_=cvar, func=Sqrt, bias=eps_t)
                nc.vector.reciprocal(out=csd, in_=csd)
                nc.gpsimd.tensor_tensor(
                    out=cn2, in0=cn2,
                    in1=cmu.unsqueeze(2).broadcast_to((128, 2, CTX)), op=SUB)
                nc.gpsimd.tensor_tensor(
                    out=cn2, in0=cn2,
                    in1=csd.unsqueeze(2).broadcast_to((128, 2, CTX)), op=MUL)
                cwv = cw_bc.rearrange("p a b -> p (a b)")
                cbv = cb_bc.rearrange("p a b -> p (a b)")
                nc.gpsimd.tensor_tensor(
                    out=cn2, in0=cn2,
                    in1=cwv.unsqueeze(1).broadcast_to((128, 2, CTX)), op=MUL)
                nc.gpsimd.tensor_tensor(
                    out=cn2, in0=cn2,
                    in1=cbv.unsqueeze(1).broadcast_to((128, 2, CTX)), op=ADD)

                # cnT[j, r, g, k] = cn[key g*128+k, ctx 6j+r]
                cnT = wides.tile([128, 6, 2, 128], F32R, tag="cnT")
                for r0 in range(0, 6, 2):
                    pt4 = pacc.tile([128, 2, 2, 128], F32, tag="acc")
                    for dr in range(2):
                        for g in range(2):
                            nc.tensor.transpose(pt4[:, dr, g, :],
                                                cn2[:, g, (r0 + dr)::6], ident)
                    nc.scalar.copy(out=cnT[:, r0:r0 + 2], in_=pt4)

                # k, v projections; kT per head; masked v_aug
                kt_all = wides.tile([128, 4, N], F32R, tag="kt_all")
                va = wides.tile([128, 2, H, D + 1], F32, tag="va")
                va_r = wides.tile([128, 2, H, 128], F32R, tag="va_r")
                for g in range(2):
                    pkv = pacc.tile([128, HID], F32, tag="acc")
                    for r in range(6):
                        nc.tensor.matmul(pkv, cnT[:, r, g, :],
                                         wkv_sb[:, r, 0:HID],
                                         start=(r == 0), stop=(r == 5))
                    ksb = work.tile([128, HID], F32, tag="ksb")
                    nc.scalar.copy(out=ksb, in_=pkv)
                    ptk = pacc.tile([128, 4, 128], F32, tag="acc")
                    for cblk in range(4):
                        nc.tensor.transpose(ptk[:, cblk, :],
                                            ksb[:, cblk * 128:(cblk + 1) * 128],
                                            ident)
                    nc.scalar.copy(out=kt_all[:, :, g * 128:(g + 1) * 128],
                                   in_=ptk)
                    pv = pacc.tile([128, HID], F32, tag="acc")
                    for r in range(6):
                        nc.tensor.matmul(pv, cnT[:, r, g, :],
                                         wkv_sb[:, r, HID:2 * HID],
                                         start=(r == 0), stop=(r == 5))
                    nc.vector.tensor_copy(
                        out=va[:, g, :, 0:D],
                        in_=pv.rearrange("p (h d) -> p h d", h=H))
                nc.vector.memset(va[:, :, :, D:D + 1], 1.0)
                for g in range(2):
                    nc.gpsimd.tensor_scalar_mul(va[:, g], va[:, g],
                                                maskf[:, g:g + 1])
                nc.vector.tensor_copy(out=va_r[:, :, :, 0:D + 1], in_=va)

                # ================= x LayerNorm + xnT ========================
                sq = persist.tile([128, GP, C], F32, tag="big2")
                nc.vector.tensor_mul(out=sq, in0=x_all, in1=x_all)
                xst = small.tile([128, GP, 2], F32, tag="xst")
                nc.vector.reduce_sum(out=xst[:, :, 0], in_=x_all,
                                     axis=mybir.AxisListType.X)
                nc.vector.reduce_sum(out=xst[:, :, 1], in_=sq,
                                     axis=mybir.AxisListType.X)
                xmu = small.tile([128, GP], F32, tag="xmu")
                nc.scalar.activation(out=xmu, in_=xst[:, :, 0], func=Copy,
                                     scale=1.0 / C)
                xvar = small.tile([128, GP], F32, tag="xvar")
                nc.vector.tensor_mul(out=xvar, in0=xmu, in1=xmu)
                nc.vector.scalar_tensor_tensor(out=xvar, in0=xst[:, :, 1],
                                               scalar=1.0 / C, in1=xvar,
                                               op0=MUL, op1=SUB)
                xsd = small.tile([128, GP], F32, tag="xsd")
                nc.scalar.activation(out=xsd, in_=xvar, func=Sqrt, bias=eps_t)
                nc.vector.reciprocal(out=xsd, in_=xsd)
                nc.gpsimd.tensor_tensor(
                    out=x_all, in0=x_all,
                    in1=xmu.unsqueeze(2).broadcast_to((128, GP, C)), op=SUB)
                nc.gpsimd.tensor_tensor(
                    out=x_all, in0=x_all,
                    in1=xsd.unsqueeze(2).broadcast_to((128, GP, C)), op=MUL)
                # xnT = transpose(z) with gamma/beta fused into the eviction
                xnT = wides.tile([C, TOK], F32R, tag="xnT")
                for t4 in range(8):
                    pt4 = pacc.tile([128, 4, 128], F32, tag="acc")
                    for j in range(4):
                        nc.tensor.transpose(pt4[:, j, :], x_all[:, 4 * t4 + j, :],
                                            ident)
                    nc.scalar.activation(
                        out=xnT[:, t4 * 512:(t4 + 1) * 512], in_=pt4,
                        func=Ident, scale=nw_col, bias=nb_col)

                # ================= attention, 512 tokens per chunk ==========
                out_sb = persist.tile([128, GP, C], F32, tag="big2")
                for t in range(NT):
                    tsl = slice(t * TCH, (t + 1) * TCH)
                    pq = pacc.tile([128, 4, TCH], F32, tag="acc")
                    for m in range(4):
                        nc.tensor.matmul(pq[:, m, :],
                                         wq_sb[:, m * 128:(m + 1) * 128],
                                         xnT[:, tsl], start=True, stop=True)
                    qT = work1.tile([128, 4, TCH], F32R, tag="qT")
                    nc.scalar.copy(out=qT, in_=pq)
                    pq5 = psim.tile([H, TCH], F32, tag="ps")
                    nc.tensor.matmul(pq5, wqnull, xnT[:, tsl],
                                     start=True, stop=True)
                    pn_exp = small.tile([H, TCH], F32R, tag="pn_exp")
                    nc.scalar.activation(out=pn_exp, in_=pq5, func=Exp,
                                         scale=SCALE)
                    outT = persist.tile([D, H, TCH], F32R, tag="outT")
                    for qd in range(2):
                        po4 = pacc.tile([D + 1, 4, TCH], F32, tag="acc")
                        for hh in range(4):
                            h = qd * 4 + hh
                            hb = (h % 2) * 64
                            qh = qT[hb:hb + 64, h // 2, :]
                            ps = psim.tile([128, 2, TCH], F32, tag="ps")
                            nc.tensor.matmul(ps[:, 0, :],
                                             kt_all[hb:hb + 64, h // 2, 0:128],
                                             qh, start=True, stop=True)
                            nc.tensor.matmul(ps[:, 1, :],
                                             kt_all[hb:hb + 64, h // 2, 128:256],
                                             qh, start=True, stop=True)
                            pe = work.tile([128, 2, TCH], F32R, tag="pexp")
                            nc.scalar.activation(out=pe, in_=ps, func=Exp,
                                                 scale=SCALE)
                            nc.tensor.matmul(po4[:, hh, :],
                                             va_r[:, 0, h, 0:D + 1],
                                             pe[:, 0, :], start=True, stop=False)
                            nc.tensor.matmul(po4[:, hh, :],
                                             va_r[:, 1, h, 0:D + 1],
                                             pe[:, 1, :], start=False, stop=False)
                            nc.tensor.matmul(po4[:, hh, :], vnull8[:, h, :],
                                             pn_exp, start=False, stop=True)
                        rc4 = work1.tile([1, 4, TCH], F32, tag="rc4")
                        nc.vector.reciprocal(out=rc4, in_=po4[D:D + 1, :, :])
                        rb4 = persist.tile([D, 4, TCH], F32, tag="rb4")
                        nc.gpsimd.partition_broadcast(rb4, rc4)
                        nc.vector.tensor_mul(out=outT[:, 4 * qd:4 * qd + 4, :],
                                             in0=po4[0:D, :, :], in1=rb4)
                    # final projection + bias, residual, transpose back
                    pf = psim.tile([C, TCH], F32, tag="ps")
                    for h in range(H):
                        nc.tensor.matmul(pf, wout_hd[:, h, :], outT[:, h, :],
                                         start=(h == 0), stop=(h == H - 1))
                    fT = work.tile([C, TCH], F32, tag="fT")
                    nc.scalar.activation(out=fT, in_=pf, func=Ident,
                                         bias=bout_col)
                    nc.gpsimd.tensor_add(out=fT, in0=fT,
                                         in1=xnT[:, tsl].bitcast(F32))
                    ptb = pacc.tile([128, 4, 128], F32, tag="acc")
                    for j in range(4):
                        nc.tensor.transpose(ptb[:, j, :],
                                            fT[:, j * 128:(j + 1) * 128], ident)
                    nc.scalar.copy(out=out_sb[:, 4 * t:4 * t + 4, :], in_=ptb)

                # one contiguous store for the whole batch element
                nc.sync.dma_start(
                    out=out_d.ap().rearrange("(p g) c -> p g c", p=128),
                    in_=out_sb)

    nc.compile()
    return nc


_CACHE = {}


def get_nc(n_iters: int = 1):
    if n_iters not in _CACHE:
        _CACHE[n_iters] = build(n_iters)
    return _CACHE[n_iters]


# ---------------------------------------------------------------------------
# Cached-executable runner: same semantics as run_bass_kernel_spmd under axon
# (bass2jax.run_bass_via_pjrt), but the jax.jit executable is built once per
# Bass module and reused across calls, skipping per-call re-trace/re-compile.
# ---------------------------------------------------------------------------
_RUNNERS = {}


def _make_runner(nc, n_cores):
    import jax
    from jax.sharding import Mesh, PartitionSpec
    from jax.experimental.shard_map import shard_map
    from concourse.bass2jax import (_bass_exec_p, install_neuronx_cc_hook,
                                    partition_id_tensor)

    install_neuronx_cc_hook()
    assert nc.dbg_addr is None
    partition_name = nc.partition_id_tensor.name if nc.partition_id_tensor else None

    in_names, out_names, out_avals, zero_shapes = [], [], [], []
    for alloc in nc.m.functions[0].allocations:
        if not isinstance(alloc, mybir.MemoryLocationSet):
            continue
        name = alloc.memorylocations[0].name
        if alloc.kind == "ExternalInput":
            if name != partition_name:
                in_names.append(name)
        elif alloc.kind == "ExternalOutput":
            shape = tuple(alloc.tensor_shape)
            dtype = mybir.dt.np(alloc.dtype)
            out_names.append(name)
            out_avals.append(jax.core.ShapedArray(shape, dtype))
            zero_shapes.append((shape, dtype))
    n_params = len(in_names)
    n_outs = len(out_avals)
    all_in_names = list(in_names) + list(out_names)
    if partition_name is not None:
        all_in_names.append(partition_name)
    donate = tuple(range(n_params, n_params + n_outs))

    def _body(*args):
        operands = list(args)
        if partition_name is not None:
            operands.append(partition_id_tensor())
        outs = _bass_exec_p.bind(
            *operands,
            out_avals=tuple(out_avals),
            in_names=tuple(all_in_names),
            out_names=tuple(out_names),
            lowering_input_output_aliases=(),
            sim_require_finite=True,
            sim_require_nnan=True,
            nc=nc,
        )
        return tuple(outs)

    devices = jax.devices()[:n_cores]
    mesh = Mesh(np.asarray(devices), ("core",))
    in_specs = (PartitionSpec("core"),) * (n_params + n_outs)
    out_specs = (PartitionSpec("core"),) * n_outs
    sharded = jax.jit(
        shard_map(_body, mesh=mesh, in_specs=in_specs, out_specs=out_specs,
                  check_rep=False),
        donate_argnums=donate, keep_unused=True,
    )

    def run(in_maps):
        per_core = [[np.asarray(m[name]) for name in in_names] for m in in_maps]
        concat_in = [
            np.concatenate([per_core[c][i] for c in range(n_cores)], axis=0)
            for i in range(n_params)
        ]
        concat_zeros = [
            np.zeros((n_cores * s[0], *s[1:]), dt) for (s, dt) in zero_shapes
        ]
        out_arrs = sharded(*concat_in, *concat_zeros)
        return [
            {
                name: np.asarray(out_arrs[i]).reshape(n_cores, *out_avals[i].shape)[c]
                for i, name in enumerate(out_names)
            }
            for c in range(n_cores)
        ]

    return run


def run_cached(nc, in_maps, n_cores=NCORES):
    key = id(nc)
    if key not in _RUNNERS:
        _RUNNERS[key] = _make_runner(nc, n_cores)
    return _RUNNERS[key](in_maps)


def make_in_maps(xt, context, mask, norm_w, norm_b, ctx_norm_w, ctx_norm_b,
                 Wq, Wkv, null_kv, Wout, bout):
    xt = np.asarray(xt, dtype=np.float32).reshape(B, TOK, C)
    context = np.asarray(context, dtype=np.float32)
    mask8 = np.asarray(mask).astype(np.uint8)
    shared = {
        "norm_w": np.asarray(norm_w, np.float32),
        "norm_b": np.asarray(norm_b, np.float32),
        "ctx_norm_w": np.asarray(ctx_norm_w, np.float32),
        "ctx_norm_b": np.asarray(ctx_norm_b, np.float32),
        "Wq": np.asarray(Wq, np.float32),
        "Wkv": np.asarray(Wkv, np.float32),
        "null_kv": np.asarray(null_kv, np.float32),
        "Wout": np.asarray(Wout, np.float32),
        "bout": np.asarray(bout, np.float32),
    }
    return [
        {"xt": xt[b], "context": context[b], "mask": mask8[b], **shared}
        for b in range(B)
    ]


def kernel(xt, context, mask, norm_w, norm_b, ctx_norm_w, ctx_norm_b,
           Wq, Wkv, null_kv, Wout, bout):
    nc = get_nc(1)
    in_maps = make_in_maps(xt, context, mask, norm_w, norm_b, ctx_norm_w,
                           ctx_norm_b, Wq, Wkv, null_kv, Wout, bout)
    res = run_cached(nc, in_maps)
    out = np.stack([res[b]["out"] for b in range(B)], axis=0)
    return out.reshape(B, XS, YS, C).astype(np.float32)
